# revision 1
# baseline (speedup 1.0000x reference)
"""BiGRU (N=64, T=512, D=512, H=512) on 8 TRN2 NeuronCores.

Sharding: data-parallel over batch (8 per core); each core runs both
directions as two interleaved GRU chains (chain0 = fwd, chain1 = bwd on
host-time-flipped x). Weights replicated (bf16), full T scan on-core.

Per chain step (batch 8):
  - gates psum [128,512] = [z_pre | r_pre | h_g | x_g]: 48 column-tiled
    matmuls (4 strips x 4 k-chunks x {W_h zrg(384), W_x zr(256), W_x g(128)}),
    stationary = h.T / x_t.T slices [128,8] bf16, moving = weight slices.
    The input projection x_t @ W_x is fused into the scan (never
    materialized in DRAM).
  - zr = sigmoid(ps[:,0:256]); g = tanh(r * ps[:,256:384] + ps[:,384:512])
  - h = g + z * (h - g)   (persistent fp32 [4 strips x 32 part, 128 units])
  - h transposed back to stationary layout with 4 col-tiled matmuls against
    a 0/1 selection matrix; fp32 copy staged to SBUF and DMA'd to the output.
"""

from contextlib import ExitStack

import numpy as np
import ml_dtypes

import concourse.bacc as bacc
import concourse.bass as bass
import concourse.tile as tile
import concourse.mybir as mybir
from concourse import bass_utils

F32 = mybir.dt.float32
BF16 = mybir.dt.bfloat16
AF = mybir.ActivationFunctionType
ALU = mybir.AluOpType

N_CORES = 8
N, T, D, H = 64, 512, 512, 512
U = 8  # time steps per DMA block / loop-body unroll


def build_gru(T_, U_, repeats=1, with_bias=False):
    assert T_ % U_ == 0
    nc = bacc.Bacc("TRN2", target_bir_lowering=False, debug=False,
                   num_devices=N_CORES)
    xs, wxs, whs, outs, bds = [], [], [], [], []
    for c in range(2):
        xs.append(nc.dram_tensor(f"x{c}", [T_ // U_, 128, U_, 4, 8], BF16,
                                 kind="ExternalInput").ap())
        wxs.append(nc.dram_tensor(f"wx{c}", [4, 128, 1536], BF16,
                                  kind="ExternalInput").ap())
        whs.append(nc.dram_tensor(f"wh{c}", [4, 128, 1536], BF16,
                                  kind="ExternalInput").ap())
        outs.append(nc.dram_tensor(f"out{c}", [T_, 128, 4, 8], F32,
                                   kind="ExternalOutput").ap())
        if with_bias:
            bds.append(nc.dram_tensor(f"b{c}", [1, 1536], BF16,
                                      kind="ExternalInput").ap())
    isel_d = nc.dram_tensor("isel", [128, 32], F32, kind="ExternalInput").ap()

    with tile.TileContext(nc) as tc, ExitStack() as ctx:
        cpool = ctx.enter_context(tc.tile_pool(name="const", bufs=1))
        xpools = [ctx.enter_context(tc.tile_pool(name=f"x{c}", bufs=2))
                  for c in range(2)]
        pspools = [ctx.enter_context(
            tc.tile_pool(name=f"ps{c}", bufs=2, space="PSUM"))
            for c in range(2)]
        ptpools = [ctx.enter_context(
            tc.tile_pool(name=f"pt{c}", bufs=2, space="PSUM"))
            for c in range(2)]
        epool = ctx.enter_context(tc.tile_pool(name="elem", bufs=3))

        isel = cpool.tile([128, 32], F32, tag="isel")
        nc.sync.dma_start(isel[:], isel_d[:])
        wx_sb, wh_sb, b_sb = [], [], []
        for c in range(2):
            wx_sb.append([cpool.tile([128, 1536], BF16, tag=f"wx{c}k{k}",
                                     name=f"wx{c}k{k}") for k in range(4)])
            wh_sb.append([cpool.tile([128, 1536], BF16, tag=f"wh{c}k{k}",
                                     name=f"wh{c}k{k}") for k in range(4)])
            for k in range(4):
                nc.sync.dma_start(wx_sb[c][k][:], wxs[c][k])
                nc.sync.dma_start(wh_sb[c][k][:], whs[c][k])
            if with_bias:
                bt = cpool.tile([1, 1536], BF16, tag=f"b{c}", name=f"b{c}")
                nc.sync.dma_start(bt[:], bds[c][:])
                b_sb.append(bt)
        if with_bias:
            ones = cpool.tile([1, 8], BF16, tag="ones")
            nc.vector.memset(ones[:], 1.0)

        h_state = [cpool.tile([128, 128], F32, tag=f"h{c}", name=f"h{c}")
                   for c in range(2)]
        hT_sb = [cpool.tile([128, 128], BF16, tag=f"hT{c}", name=f"hT{c}")
                 for c in range(2)]
        for c in range(2):
            nc.vector.memset(h_state[c][:], 0.0)
            nc.vector.memset(hT_sb[c][:], 0.0)

        def emit_step(c, x_tile, tl, t_dyn):
            ps = pspools[c].tile([128, 512], F32, tag=f"ps{c}", name="ps")
            for j in range(4):
                # One accumulation group per strip per step (psum start/stop
                # act on the whole 2KB bank per partition): first MM starts,
                # last MM stops, everything else accumulates.
                mms = []
                for k in range(4):
                    sh = hT_sb[c][:, 32 * k:32 * k + 8]
                    sx = x_tile[:, (tl * 4 + k) * 8:(tl * 4 + k) * 8 + 8]
                    mms.append((ps[32 * j:32 * j + 8, 0:384], sh,
                                wh_sb[c][k][:, 384 * j:384 * j + 384]))
                    mms.append((ps[32 * j:32 * j + 8, 0:256], sx,
                                wx_sb[c][k][:, 384 * j:384 * j + 256]))
                    mms.append((ps[32 * j:32 * j + 8, 384:512], sx,
                                wx_sb[c][k][:, 384 * j + 256:384 * j + 384]))
                if with_bias:
                    mms.append((ps[32 * j:32 * j + 8, 0:256], ones[:],
                                b_sb[c][:, 384 * j:384 * j + 256]))
                    mms.append((ps[32 * j:32 * j + 8, 384:512], ones[:],
                                b_sb[c][:, 384 * j + 256:384 * j + 384]))
                for idx, (o, lt, rh) in enumerate(mms):
                    nc.tensor.matmul(o, lhsT=lt, rhs=rh,
                                     start=(idx == 0),
                                     stop=(idx == len(mms) - 1),
                                     tile_position=(0, 32 * j))

            zr = epool.tile([128, 256], F32, tag=f"zr{c}", name="zr")
            nc.scalar.activation(zr[:], ps[:, 0:256], AF.Sigmoid)
            t1 = epool.tile([128, 128], F32, tag=f"t1{c}", name="t1")
            nc.vector.tensor_tensor(t1[:], zr[:, 128:256], ps[:, 256:384],
                                    ALU.mult)
            gp = epool.tile([128, 128], F32, tag=f"gp{c}", name="gp")
            nc.vector.tensor_tensor(gp[:], t1[:], ps[:, 384:512], ALU.add)
            g = epool.tile([128, 128], F32, tag=f"g{c}", name="g")
            nc.scalar.activation(g[:], gp[:], AF.Tanh)
            dtl = epool.tile([128, 128], F32, tag=f"d{c}", name="dtl")
            nc.vector.tensor_tensor(dtl[:], h_state[c][:], g[:], ALU.subtract)
            m = epool.tile([128, 128], F32, tag=f"m{c}", name="m")
            nc.vector.tensor_tensor(m[:], zr[:, 0:128], dtl[:], ALU.mult)
            nc.vector.tensor_tensor(h_state[c][:], m[:], g[:], ALU.add)

            pt = ptpools[c].tile([128, 32], F32, tag=f"pt{c}", name="pt")
            for mb in range(4):
                nc.tensor.matmul(
                    pt[32 * mb:32 * mb + 32, :],
                    lhsT=h_state[c][:, 32 * mb:32 * mb + 32],
                    rhs=isel[:], start=True, stop=True,
                    tile_position=(0, 32 * mb))
            hT_view = hT_sb[c][:].rearrange("p (k w) -> p k w", k=4)
            nc.vector.tensor_copy(
                hT_view[:, :, 0:8],
                pt[:].rearrange("p (s b) -> p s b", s=4))
            hTf = epool.tile([128, 32], F32, tag=f"hTf{c}", name="hTf")
            nc.scalar.copy(hTf[:], pt[:])
            dst = outs[c][bass.ds(t_dyn, 1)].rearrange(
                "o p s b -> (o p) s b")
            nc.sync.dma_start(dst, hTf[:].rearrange("p (s b) -> p s b", s=4))

        def time_block(i_dyn):
            x_tiles = []
            for c in range(2):
                xt = xpools[c].tile([128, U_ * 32], BF16, tag=f"xt{c}",
                                    name=f"xt{c}")
                src = xs[c][bass.ds(i_dyn, 1)].rearrange(
                    "o p u k n -> (o p) (u k n)")
                nc.sync.dma_start(xt[:], src)
                x_tiles.append(xt)
            for tl in range(U_):
                for c in range(2):
                    emit_step(c, x_tiles[c], tl, i_dyn * U_ + tl)

        n_blocks = T_ // U_
        if repeats == 1:
            with tc.For_i(0, n_blocks) as i:
                time_block(i)
        else:
            with tc.For_i(0, repeats) as rr:
                with tc.For_i(0, n_blocks) as i:
                    time_block(i)
    nc.compile()
    return nc


def arrange_w(w):
    """[512, 1536] -> [4, 128, 1536]: k-chunk, d', strip-major [z|r|g]."""
    w = np.asarray(w, np.float32).reshape(4, 128, 3, 4, 128)
    w = w.transpose(0, 1, 3, 2, 4).reshape(4, 128, 1536)
    return np.ascontiguousarray(w).astype(ml_dtypes.bfloat16)


def arrange_b(b):
    b = np.asarray(b, np.float32).reshape(3, 4, 128).transpose(1, 0, 2)
    return np.ascontiguousarray(b.reshape(1, 1536)).astype(ml_dtypes.bfloat16)


def arrange_x_all(x, U_):
    """[N, T, D] f32 -> [T//U, 128, U, 4, N] bf16 (slice batch last)."""
    n, t, _ = x.shape
    xt = np.transpose(x, (1, 2, 0)).reshape(t // U_, U_, 4, 128, n)
    return np.ascontiguousarray(xt.transpose(0, 3, 1, 2, 4)).astype(
        ml_dtypes.bfloat16)


def make_isel():
    isel = np.zeros((128, 32), np.float32)
    for s in range(4):
        for b in range(8):
            isel[32 * s + b, 8 * s + b] = 1.0
    return isel


def decode_out(o):
    """[T, 128, 4, 8] -> [8, T, 512] via h[b,t,128s+p] = o[t,p,s,b]."""
    t = o.shape[0]
    return np.ascontiguousarray(o.transpose(3, 0, 2, 1).reshape(8, t, 512))


_CACHE = {}


def _get_program(with_bias):
    key = ("prog", with_bias)
    if key not in _CACHE:
        _CACHE[key] = build_gru(T, U, repeats=1, with_bias=with_bias)
    return _CACHE[key]


def kernel(x, W_x_fwd, W_h_fwd, b_fwd, W_x_bwd, W_h_bwd, b_bwd):
    x = np.asarray(x, np.float32)
    W_x_fwd = np.asarray(W_x_fwd, np.float32)
    W_h_fwd = np.asarray(W_h_fwd, np.float32)
    W_x_bwd = np.asarray(W_x_bwd, np.float32)
    W_h_bwd = np.asarray(W_h_bwd, np.float32)
    b_fwd = np.asarray(b_fwd, np.float32)
    b_bwd = np.asarray(b_bwd, np.float32)
    assert x.shape == (N, T, D), x.shape

    with_bias = bool(np.any(b_fwd) or np.any(b_bwd))
    nc = _get_program(with_bias)

    x_fwd = arrange_x_all(x, U)                  # [T//U,128,U,4,64]
    x_bwd = arrange_x_all(x[:, ::-1], U)
    base = {
        "wx0": arrange_w(W_x_fwd), "wh0": arrange_w(W_h_fwd),
        "wx1": arrange_w(W_x_bwd), "wh1": arrange_w(W_h_bwd),
        "isel": make_isel(),
    }
    if with_bias:
        base["b0"] = arrange_b(b_fwd)
        base["b1"] = arrange_b(b_bwd)
    in_maps = []
    for c in range(N_CORES):
        m = dict(base)
        m["x0"] = np.ascontiguousarray(x_fwd[..., 8 * c:8 * c + 8])
        m["x1"] = np.ascontiguousarray(x_bwd[..., 8 * c:8 * c + 8])
        in_maps.append(m)

    res = bass_utils.run_bass_kernel_spmd(nc, in_maps,
                                          core_ids=list(range(N_CORES)))
    out = np.empty((N, T, 2 * H), np.float32)
    for c in range(N_CORES):
        sl = slice(8 * c, 8 * c + 8)
        out[sl, :, :H] = decode_out(res.results[c]["out0"])
        out[sl, :, H:] = decode_out(res.results[c]["out1"])[:, ::-1]
    return out



# revision 10
# speedup vs baseline: 1.7591x; 1.7591x over previous
"""BiGRU (N=64, T=512, D=512, H=512) on 8 TRN2 NeuronCores.

Sharding: data-parallel over batch (8 per core); each core runs both
directions as two interleaved GRU chains (chain0 = fwd, chain1 = bwd on
host-time-flipped x). Weights replicated (bf16), full T scan on-core.

Per chain step (batch 8):
  - gates psum [128,512] = [z_pre | r_pre | h_g | x_g]: 48 column-tiled
    matmuls (4 strips x 4 k-chunks x {W_h zrg(384), W_x zr(256), W_x g(128)}),
    stationary = h.T / x_t.T slices [128,8] bf16, moving = weight slices.
    The input projection x_t @ W_x is fused into the scan (never
    materialized in DRAM).
  - zr = sigmoid(ps[:,0:256]); g = tanh(r * ps[:,256:384] + ps[:,384:512])
  - h = g + z * (h - g)   (persistent fp32 [4 strips x 32 part, 128 units])
  - h transposed back to stationary layout with 4 col-tiled matmuls against
    a 0/1 selection matrix; fp32 copy staged to SBUF and DMA'd to the output.
"""

from contextlib import ExitStack

import numpy as np
import ml_dtypes

import concourse.bacc as bacc
import concourse.bass as bass
import concourse.tile as tile
import concourse.mybir as mybir
from concourse import bass_utils

F32 = mybir.dt.float32
BF16 = mybir.dt.bfloat16
AF = mybir.ActivationFunctionType
ALU = mybir.AluOpType

N_CORES = 8
N, T, D, H = 64, 512, 512, 512
U = 8  # time steps per DMA block / loop-body unroll


def build_gru(T_, U_, repeats=1, with_bias=False, sim_init=False):
    assert T_ % U_ == 0
    nc = bacc.Bacc("TRN2", target_bir_lowering=False, debug=False,
                   num_devices=N_CORES)
    xs, wxs, whs, outs, bds = [], [], [], [], []
    for c in range(2):
        xs.append(nc.dram_tensor(f"x{c}", [T_ // U_, 128, U_, 4, 8], BF16,
                                 kind="ExternalInput").ap())
        wxs.append(nc.dram_tensor(f"wx{c}", [4, 128, 1536], BF16,
                                  kind="ExternalInput").ap())
        whs.append(nc.dram_tensor(f"wh{c}", [4, 128, 1536], BF16,
                                  kind="ExternalInput").ap())
        outs.append(nc.dram_tensor(f"out{c}", [T_, 128, 4, 8], F32,
                                   kind="ExternalOutput").ap())
        if with_bias:
            bds.append(nc.dram_tensor(f"b{c}", [1, 1536], BF16,
                                      kind="ExternalInput").ap())
    isel_d = nc.dram_tensor("isel", [128, 32], F32, kind="ExternalInput").ap()

    with tile.TileContext(nc) as tc, ExitStack() as ctx:
        cpool = ctx.enter_context(tc.tile_pool(name="const", bufs=1))
        xpools = [ctx.enter_context(tc.tile_pool(name=f"x{c}", bufs=2))
                  for c in range(2)]
        pspools = [ctx.enter_context(
            tc.tile_pool(name=f"ps{c}", bufs=2, space="PSUM"))
            for c in range(2)]
        ptpools = [ctx.enter_context(
            tc.tile_pool(name=f"pt{c}", bufs=2, space="PSUM"))
            for c in range(2)]
        epool = ctx.enter_context(tc.tile_pool(name="elem", bufs=3))

        isel = cpool.tile([128, 32], F32, tag="isel")
        nc.sync.dma_start(isel[:], isel_d[:])
        wx_sb, wh_sb, b_sb = [], [], []
        for c in range(2):
            wx_sb.append([cpool.tile([128, 1536], BF16, tag=f"wx{c}k{k}",
                                     name=f"wx{c}k{k}") for k in range(4)])
            wh_sb.append([cpool.tile([128, 1536], BF16, tag=f"wh{c}k{k}",
                                     name=f"wh{c}k{k}") for k in range(4)])
            for k in range(4):
                nc.sync.dma_start(wx_sb[c][k][:], wxs[c][k])
                nc.sync.dma_start(wh_sb[c][k][:], whs[c][k])
            if with_bias:
                bt = cpool.tile([1, 1536], BF16, tag=f"b{c}", name=f"b{c}")
                nc.sync.dma_start(bt[:], bds[c][:])
                b_sb.append(bt)
        if with_bias:
            ones = cpool.tile([1, 8], BF16, tag="ones")
            nc.vector.memset(ones[:], 1.0)

        h_state = [cpool.tile([128, 128], F32, tag=f"h{c}", name=f"h{c}")
                   for c in range(2)]
        hT_sb = [cpool.tile([128, 128], BF16, tag=f"hT{c}", name=f"hT{c}")
                 for c in range(2)]
        for c in range(2):
            nc.vector.memset(h_state[c][:], 0.0)
            nc.vector.memset(hT_sb[c][:], 0.0)

        def emit_step(c, x_tile, tl, t_dyn):
            ps = pspools[c].tile([128, 512], F32, tag=f"ps{c}", name="ps")
            if sim_init:
                # TimelineSim-only: lead each strip group with an M=32,
                # N=512 matmul that fully covers the strip's psum rows so
                # the interpreter never reads uninitialized memory (HW
                # reads junk there harmlessly).
                for j in range(4):
                    nc.tensor.matmul(ps[32 * j:32 * j + 32, 0:512],
                                     lhsT=hT_sb[c][:, 0:32],
                                     rhs=wh_sb[c][0][:, 0:512],
                                     start=True, stop=False,
                                     tile_position=(0, 32 * j))
            # One accumulation group per strip per step (start/stop act on
            # the strip's own psum partitions). Emit the matmuls
            # column-group-INTERLEAVED: PE starts matmuls in strict program
            # order, so back-to-back matmuls on the same col-group
            # serialize; rotating across the 4 col-groups lets all 4 run
            # concurrently (tile-packing, ~4ns stagger).
            mms = [[] for _ in range(4)]
            for j in range(4):
                for k in range(4):
                    sh = hT_sb[c][:, 32 * k:32 * k + 8]
                    sx = x_tile[:, (tl * 4 + k) * 8:(tl * 4 + k) * 8 + 8]
                    mms[j].append((ps[32 * j:32 * j + 8, 0:384], sh,
                                   wh_sb[c][k][:, 384 * j:384 * j + 384]))
                    mms[j].append((ps[32 * j:32 * j + 8, 0:256], sx,
                                   wx_sb[c][k][:, 384 * j:384 * j + 256]))
                    mms[j].append((ps[32 * j:32 * j + 8, 384:512], sx,
                                   wx_sb[c][k][:, 384 * j + 256:384 * j + 384]))
                if with_bias:
                    mms[j].append((ps[32 * j:32 * j + 8, 0:256], ones[:],
                                   b_sb[c][:, 384 * j:384 * j + 256]))
                    mms[j].append((ps[32 * j:32 * j + 8, 384:512], ones[:],
                                   b_sb[c][:, 384 * j + 256:384 * j + 384]))
            n_mm = len(mms[0])
            for idx in range(n_mm):
                for j in range(4):
                    o, lt, rh = mms[j][idx]
                    nc.tensor.matmul(o, lhsT=lt, rhs=rh,
                                     start=(idx == 0 and not sim_init),
                                     stop=(idx == n_mm - 1 and not sim_init),
                                     tile_position=(0, 32 * j))
            if sim_init:
                # Close the 32-partition groups opened by the init MMs
                # (N=1: negligible cost, clears the whole zero region).
                for j in range(4):
                    nc.tensor.matmul(ps[32 * j:32 * j + 32, 0:1],
                                     lhsT=hT_sb[c][:, 0:32],
                                     rhs=wh_sb[c][0][:, 0:1],
                                     start=False, stop=True,
                                     tile_position=(0, 32 * j))

            zr = epool.tile([128, 256], F32, tag=f"zr{c}", name="zr")
            nc.scalar.activation(zr[:], ps[:, 0:256], AF.Sigmoid)
            t1 = epool.tile([128, 128], F32, tag=f"t1{c}", name="t1")
            nc.vector.tensor_tensor(t1[:], zr[:, 128:256], ps[:, 256:384],
                                    ALU.mult)
            gp = epool.tile([128, 128], F32, tag=f"gp{c}", name="gp")
            nc.vector.tensor_tensor(gp[:], t1[:], ps[:, 384:512], ALU.add)
            g = epool.tile([128, 128], F32, tag=f"g{c}", name="g")
            nc.scalar.activation(g[:], gp[:], AF.Tanh)
            dtl = epool.tile([128, 128], F32, tag=f"d{c}", name="dtl")
            nc.vector.tensor_tensor(dtl[:], h_state[c][:], g[:], ALU.subtract)
            m = epool.tile([128, 128], F32, tag=f"m{c}", name="m")
            nc.vector.tensor_tensor(m[:], zr[:, 0:128], dtl[:], ALU.mult)
            nc.vector.tensor_tensor(h_state[c][:], m[:], g[:], ALU.add)

            pt = ptpools[c].tile([128, 32], F32, tag=f"pt{c}", name="pt")
            for mb in range(4):
                nc.tensor.matmul(
                    pt[32 * mb:32 * mb + 32, :],
                    lhsT=h_state[c][:, 32 * mb:32 * mb + 32],
                    rhs=isel[:], start=True, stop=True,
                    tile_position=(0, 32 * mb),
                    skip_group_check=sim_init)
            hT_view = hT_sb[c][:].rearrange("p (k w) -> p k w", k=4)
            nc.vector.tensor_copy(
                hT_view[:, :, 0:8],
                pt[:].rearrange("p (s b) -> p s b", s=4))
            hTf = epool.tile([128, 32], F32, tag=f"hTf{c}", name="hTf")
            nc.scalar.copy(hTf[:], pt[:])
            dst = outs[c][bass.ds(t_dyn, 1)].rearrange(
                "o p s b -> (o p) s b")
            nc.sync.dma_start(dst, hTf[:].rearrange("p (s b) -> p s b", s=4))

        def time_block(i_dyn):
            x_tiles = []
            for c in range(2):
                xt = xpools[c].tile([128, U_ * 32], BF16, tag=f"xt{c}",
                                    name=f"xt{c}")
                src = xs[c][bass.ds(i_dyn, 1)].rearrange(
                    "o p u k n -> (o p) (u k n)")
                nc.sync.dma_start(xt[:], src)
                x_tiles.append(xt)
            for tl in range(U_):
                for c in range(2):
                    emit_step(c, x_tiles[c], tl, i_dyn * U_ + tl)

        n_blocks = T_ // U_
        if repeats == 1:
            with tc.For_i(0, n_blocks) as i:
                time_block(i)
        else:
            with tc.For_i(0, repeats) as rr:
                with tc.For_i(0, n_blocks) as i:
                    time_block(i)
    nc.compile()
    return nc


def arrange_w(w):
    """[512, 1536] -> [4, 128, 1536]: k-chunk, d', strip-major [z|r|g]."""
    w = np.asarray(w, np.float32).reshape(4, 128, 3, 4, 128)
    w = w.transpose(0, 1, 3, 2, 4).reshape(4, 128, 1536)
    return np.ascontiguousarray(w).astype(ml_dtypes.bfloat16)


def arrange_b(b):
    b = np.asarray(b, np.float32).reshape(3, 4, 128).transpose(1, 0, 2)
    return np.ascontiguousarray(b.reshape(1, 1536)).astype(ml_dtypes.bfloat16)


def arrange_x_all(x, U_):
    """[N, T, D] f32 -> [T//U, 128, U, 4, N] bf16 (slice batch last)."""
    n, t, _ = x.shape
    xt = np.transpose(x, (1, 2, 0)).reshape(t // U_, U_, 4, 128, n)
    return np.ascontiguousarray(xt.transpose(0, 3, 1, 2, 4)).astype(
        ml_dtypes.bfloat16)


def make_isel():
    isel = np.zeros((128, 32), np.float32)
    for s in range(4):
        for b in range(8):
            isel[32 * s + b, 8 * s + b] = 1.0
    return isel


def decode_out(o):
    """[T, 128, 4, 8] -> [8, T, 512] via h[b,t,128s+p] = o[t,p,s,b]."""
    t = o.shape[0]
    return np.ascontiguousarray(o.transpose(3, 0, 2, 1).reshape(8, t, 512))


_CACHE = {}


def _get_program(with_bias):
    key = ("prog", with_bias)
    if key not in _CACHE:
        _CACHE[key] = build_gru(T, U, repeats=1, with_bias=with_bias)
    return _CACHE[key]


def kernel(x, W_x_fwd, W_h_fwd, b_fwd, W_x_bwd, W_h_bwd, b_bwd):
    x = np.asarray(x, np.float32)
    W_x_fwd = np.asarray(W_x_fwd, np.float32)
    W_h_fwd = np.asarray(W_h_fwd, np.float32)
    W_x_bwd = np.asarray(W_x_bwd, np.float32)
    W_h_bwd = np.asarray(W_h_bwd, np.float32)
    b_fwd = np.asarray(b_fwd, np.float32)
    b_bwd = np.asarray(b_bwd, np.float32)
    assert x.shape == (N, T, D), x.shape

    with_bias = bool(np.any(b_fwd) or np.any(b_bwd))
    nc = _get_program(with_bias)

    x_fwd = arrange_x_all(x, U)                  # [T//U,128,U,4,64]
    x_bwd = arrange_x_all(x[:, ::-1], U)
    base = {
        "wx0": arrange_w(W_x_fwd), "wh0": arrange_w(W_h_fwd),
        "wx1": arrange_w(W_x_bwd), "wh1": arrange_w(W_h_bwd),
        "isel": make_isel(),
    }
    if with_bias:
        base["b0"] = arrange_b(b_fwd)
        base["b1"] = arrange_b(b_bwd)
    in_maps = []
    for c in range(N_CORES):
        m = dict(base)
        m["x0"] = np.ascontiguousarray(x_fwd[..., 8 * c:8 * c + 8])
        m["x1"] = np.ascontiguousarray(x_bwd[..., 8 * c:8 * c + 8])
        in_maps.append(m)

    res = bass_utils.run_bass_kernel_spmd(nc, in_maps,
                                          core_ids=list(range(N_CORES)))
    out = np.empty((N, T, 2 * H), np.float32)
    for c in range(N_CORES):
        sl = slice(8 * c, 8 * c + 8)
        out[sl, :, :H] = decode_out(res.results[c]["out0"])
        out[sl, :, H:] = decode_out(res.results[c]["out1"])[:, ::-1]
    return out



# revision 11
# speedup vs baseline: 1.7694x; 1.0059x over previous
"""BiGRU (N=64, T=512, D=512, H=512) on 8 TRN2 NeuronCores, v2.

Sharding: direction-parallel x batch-parallel. Cores 0-3 run the forward
GRU on seqs [16c, 16c+16); cores 4-7 run the backward GRU (host-flipped x)
on seqs [16(c-4), 16(c-4)+16). One batch-16 chain per core: the scan is
latency/PE-serial-bound, so a second chain per core would only add queueing.

Phase 1 (proj): xm[t, b, :] = x[b, t, :] @ W_x (+ b) as efficient M=128
matmuls (stationary = x-chunk [128d, 128t]), staged to DRAM in bf16,
strip-major gate order [z|r|g] per strip.

Phase 2 (scan), per step, per strip j (4 col-groups, emission interleaved):
  psum layout [hg | z | r | xg] (128 cols each):
    inject: 1 MM  ps[.., 128:512] += I16.T @ xm[t]      (N=384)
    W_h:    4 MMs ps[.., 0:384]   += hT_k.T @ Wh_k      (N=384, [g|z|r] cols)
  zr = sigmoid(ps[:,128:384]) [ACT, bf16]; zc = sigmoid(-ps[:,128:256]) [ACT]
  t1 = zr_r * ps_hg; gp = t1 + ps_xg [DVE]; g = tanh(gp) [ACT]
  a = z*h_prev; b = zc*g; h' = a + b -> out-block slice [DVE, bf16]
  transpose h' via 4 isel matmuls -> psum; copy -> hT_sb (stationary for t+1)
Out blocks (U=8 steps) DMA to DRAM bf16; host decodes/casts to f32.
"""

from contextlib import ExitStack

import numpy as np
import ml_dtypes

import concourse.bacc as bacc
import concourse.bass as bass
import concourse.tile as tile
import concourse.mybir as mybir
from concourse import bass_utils

F32 = mybir.dt.float32
BF16 = mybir.dt.bfloat16
AF = mybir.ActivationFunctionType
ALU = mybir.AluOpType

N_CORES = 8
N, T, D, H = 64, 512, 512, 512
B = 16   # sequences per core
U = 8    # scan steps per block


def build_gru2(T_, U_, repeats=1, with_bias=False, sim_init=False,
               halves=True):
    assert T_ % U_ == 0
    nc = bacc.Bacc("TRN2", target_bir_lowering=False, debug=False,
                   num_devices=N_CORES)
    xin = nc.dram_tensor("xin", [B, 4, 128, T_], BF16,
                         kind="ExternalInput").ap()
    wx_d = nc.dram_tensor("wx", [4, 128, 1536], BF16,
                          kind="ExternalInput").ap()
    wh_d = nc.dram_tensor("wh", [4, 128, 1536], BF16,
                          kind="ExternalInput").ap()
    isel_d = nc.dram_tensor("isel", [128, 64], BF16,
                            kind="ExternalInput").ap()
    i16_d = nc.dram_tensor("i16", [16, 16], BF16, kind="ExternalInput").ap()
    xm_d = nc.dram_tensor("xm", [T_, B, 1536], BF16, kind="Internal").ap()
    out_d = nc.dram_tensor("out", [T_ // U_, 128, U_ * 128], BF16,
                           kind="ExternalOutput").ap()
    if with_bias:
        b_d = nc.dram_tensor("b", [1, 1536], BF16, kind="ExternalInput").ap()

    n_tb = T_ // 128          # 128-step t-tiles per sequence
    with tile.TileContext(nc) as tc, ExitStack() as ctx:
        cpool = ctx.enter_context(tc.tile_pool(name="const", bufs=1))
        xpool = ctx.enter_context(tc.tile_pool(name="xp", bufs=2))
        ppool = ctx.enter_context(tc.tile_pool(name="pp", bufs=2,
                                               space="PSUM"))
        pmpool = ctx.enter_context(tc.tile_pool(name="pm", bufs=2))
        smpool = ctx.enter_context(tc.tile_pool(name="sm", bufs=2))
        pspool = ctx.enter_context(tc.tile_pool(name="ps", bufs=2,
                                                space="PSUM"))
        ptpool = ctx.enter_context(tc.tile_pool(name="pt", bufs=2,
                                                space="PSUM"))
        epool = ctx.enter_context(tc.tile_pool(name="elem", bufs=3))
        opool = ctx.enter_context(tc.tile_pool(name="ob", bufs=2))

        wx_sb = [cpool.tile([128, 1536], BF16, tag=f"wx{k}", name=f"wx{k}")
                 for k in range(4)]
        wh_sb = [cpool.tile([128, 1536], BF16, tag=f"wh{k}", name=f"wh{k}")
                 for k in range(4)]
        for k in range(4):
            nc.sync.dma_start(wx_sb[k][:], wx_d[k])
            nc.sync.dma_start(wh_sb[k][:], wh_d[k])
        isel = cpool.tile([128, 64], BF16, tag="isel")
        nc.sync.dma_start(isel[:], isel_d[:])
        i16 = cpool.tile([16, 16], BF16, tag="i16")
        nc.sync.dma_start(i16[:], i16_d[:])
        if with_bias:
            b_sb = cpool.tile([1, 1536], BF16, tag="b")
            nc.sync.dma_start(b_sb[:], b_d[:])
            ones = cpool.tile([1, 128], BF16, tag="ones")
            nc.vector.memset(ones[:], 1.0)

        h_carry = cpool.tile([128, 128], BF16, tag="hc", name="hc")
        hT_sb = cpool.tile([128, 64], BF16, tag="hT", name="hT")

        def emit_all():
            nc.vector.memset(h_carry[:], 0.0)
            nc.vector.memset(hT_sb[:], 0.0)

            # ---------- phase 1: x @ W_x -> xm (DRAM) ----------
            xmv = xm_d.rearrange("t b f -> b t f")
            with tc.For_i(0, B) as bi:
                xt = xpool.tile([128, 4, T_], BF16, tag="xt", name="xt")
                src = xin[bass.ds(bi, 1)].rearrange("o k p t -> (o p) k t")
                nc.sync.dma_start(xt[:], src)
                xmb = xmv[bass.ds(bi, 1)]          # [1, T, 1536]
                for m in range(n_tb):
                    pm = pmpool.tile([128, 1536], BF16, tag="pm", name="pm")
                    for q in range(3):
                        pp = ppool.tile([128, 512], F32, tag="pp", name="pp")
                        nmm = 4 + (1 if with_bias else 0)
                        for k in range(4):
                            nc.tensor.matmul(
                                pp[:], lhsT=xt[:, k, 128 * m:128 * m + 128],
                                rhs=wx_sb[k][:, 512 * q:512 * q + 512],
                                start=(k == 0), stop=(k == nmm - 1))
                        if with_bias:
                            nc.tensor.matmul(
                                pp[:], lhsT=ones[:],
                                rhs=b_sb[:, 512 * q:512 * q + 512],
                                start=False, stop=True)
                        nc.vector.tensor_copy(pm[:, 512 * q:512 * q + 512],
                                              pp[:])
                    dst = xmb[:, 128 * m:128 * m + 128, :].rearrange(
                        "o t f -> (o t) f")
                    nc.sync.dma_start(dst, pm[:])

            # ---------- phase 2: the scan ----------
            def emit_step(xm_t, ot, tl, hprev):
                ps = pspool.tile([128, 512], F32, tag="ps", name="ps")
                if sim_init:
                    for j in range(4):
                        nc.tensor.matmul(ps[32 * j:32 * j + 32, 0:512],
                                         lhsT=isel[:, 0:32],
                                         rhs=wh_sb[0][:, 0:512],
                                         start=True, stop=False,
                                         tile_position=(0, 32 * j),
                                         skip_group_check=True)
                # per strip j: group = inject (N=384, cols 128:512) then
                # 4 x W_h (N=384, cols 0:384); col-group interleaved.
                mms = [[] for _ in range(4)]
                for j in range(4):
                    mms[j].append((ps[32 * j:32 * j + 16, 128:512], i16[:],
                                   xm_t[:, tl, 384 * j:384 * j + 384]))
                    for k in range(4):
                        mms[j].append(
                            (ps[32 * j:32 * j + 16, 0:384],
                             hT_sb[:, 16 * k:16 * k + 16],
                             wh_sb[k][:, 384 * j:384 * j + 384]))
                for idx in range(5):
                    for j in range(4):
                        o, lt, rh = mms[j][idx]
                        nc.tensor.matmul(o, lhsT=lt, rhs=rh,
                                         start=(idx == 0 and not sim_init),
                                         stop=(idx == 4),
                                         tile_position=(0, 32 * j),
                                         skip_group_check=sim_init)

                zr = epool.tile([128, 256], BF16, tag="zr", name="zr")
                nc.scalar.activation(zr[:], ps[:, 128:384], AF.Sigmoid)
                zc = epool.tile([128, 128], BF16, tag="zc", name="zc")
                nc.scalar.activation(zc[:], ps[:, 128:256], AF.Sigmoid,
                                     scale=-1.0)
                t1 = epool.tile([128, 128], BF16, tag="t1", name="t1")
                nc.vector.tensor_tensor(t1[:], zr[:, 128:256], ps[:, 0:128],
                                        ALU.mult)
                gp = epool.tile([128, 128], BF16, tag="gp", name="gp")
                nc.vector.tensor_tensor(gp[:], t1[:], ps[:, 384:512],
                                        ALU.add)
                g = epool.tile([128, 128], BF16, tag="g", name="g")
                nc.scalar.activation(g[:], gp[:], AF.Tanh)
                a = epool.tile([128, 128], BF16, tag="a", name="a")
                nc.vector.tensor_tensor(a[:], zr[:, 0:128], hprev, ALU.mult)
                b2 = epool.tile([128, 128], BF16, tag="b2", name="b2")
                nc.vector.tensor_tensor(b2[:], zc[:], g[:], ALU.mult)
                hs = ot[:].rearrange("p (u f) -> p u f", u=U_)
                nc.vector.tensor_tensor(hs[:, tl, :], a[:], b2[:], ALU.add)

                pt = ptpool.tile([128, 64], F32, tag="pt", name="pt")
                for mb in range(4):
                    nc.tensor.matmul(
                        pt[32 * mb:32 * mb + 32, :],
                        lhsT=hs[:, tl, 32 * mb:32 * mb + 32],
                        rhs=isel[:], start=True, stop=True,
                        tile_position=(0, 32 * mb),
                        skip_group_check=sim_init)
                nc.vector.tensor_copy(hT_sb[:], pt[:])

            with tc.For_i(0, T_ // U_) as i:
                xm_t = smpool.tile([B, U_, 1536], BF16, tag="xmt",
                                   name="xmt")
                src = xm_d[bass.ds(i * U_, U_)].rearrange("u b f -> b u f")
                nc.sync.dma_start(xm_t[:], src)
                ot = opool.tile([128, U_ * 128], BF16, tag="ot", name="ot")
                otv = ot[:].rearrange("p (u f) -> p u f", u=U_)
                for tl in range(U_):
                    hprev = h_carry[:] if tl == 0 else otv[:, tl - 1, :]
                    emit_step(xm_t, ot, tl, hprev)
                nc.vector.tensor_copy(h_carry[:], otv[:, U_ - 1, :])
                dst = out_d[bass.ds(i, 1)].rearrange("o p f -> (o p) f")
                nc.sync.dma_start(dst, ot[:])

        if repeats == 1:
            emit_all()
        else:
            with tc.For_i(0, repeats):
                emit_all()
    nc.compile()
    return nc


def arrange_w(w, gate_order=(0, 1, 2)):
    """[512, 1536] -> [4, 128, 1536]: k-chunk, d', strip-major gates."""
    w = np.asarray(w, np.float32).reshape(4, 128, 3, 4, 128)
    w = w[:, :, gate_order, :, :]
    w = w.transpose(0, 1, 3, 2, 4).reshape(4, 128, 1536)
    return np.ascontiguousarray(w).astype(ml_dtypes.bfloat16)


def arrange_b(b):
    b = np.asarray(b, np.float32).reshape(3, 4, 128).transpose(1, 0, 2)
    return np.ascontiguousarray(b.reshape(1, 1536)).astype(ml_dtypes.bfloat16)


def arrange_x(x):
    """[B, T, D] f32 -> [B, 4, 128, T] bf16."""
    xt = np.transpose(np.asarray(x, np.float32), (0, 2, 1))
    xt = xt.reshape(B, 4, 128, -1)
    return np.ascontiguousarray(xt).astype(ml_dtypes.bfloat16)


def make_isel16():
    m = np.zeros((128, 64), np.float32)
    for s in range(4):
        for b in range(B):
            m[32 * s + b, 16 * s + b] = 1.0
    return m.astype(ml_dtypes.bfloat16)


def decode_out(o, T_):
    """[T/U, 128, U*128] bf16 -> [B, T, 512] f32.

    o[i, 32s+b, 128u+f] = h[b, U*i+u, 128s+f]  (b < 16)."""
    o = np.asarray(o, np.float32).reshape(T_ // U, 4, 32, U, 128)
    o = o[:, :, :B]                       # [i, s, b, u, f]
    o = o.transpose(2, 0, 3, 1, 4)        # [b, i, u, s, f]
    return np.ascontiguousarray(o.reshape(B, T_, 512))


_CACHE = {}


def _get_program(with_bias):
    key = ("prog2", with_bias)
    if key not in _CACHE:
        _CACHE[key] = build_gru2(T, U, repeats=1, with_bias=with_bias)
    return _CACHE[key]


def make_in_maps(x, W_x_fwd, W_h_fwd, b_fwd, W_x_bwd, W_h_bwd, b_bwd,
                 with_bias):
    x = np.asarray(x, np.float32)
    xf = x
    xb = x[:, ::-1]
    base_f = {
        "wx": arrange_w(W_x_fwd),
        "wh": arrange_w(W_h_fwd, gate_order=(2, 0, 1)),  # [g|z|r]
        "isel": make_isel16(),
        "i16": np.eye(16, dtype=np.float32).astype(ml_dtypes.bfloat16),
    }
    base_b = {
        "wx": arrange_w(W_x_bwd),
        "wh": arrange_w(W_h_bwd, gate_order=(2, 0, 1)),
        "isel": base_f["isel"],
        "i16": base_f["i16"],
    }
    if with_bias:
        base_f["b"] = arrange_b(b_fwd)
        base_b["b"] = arrange_b(b_bwd)
    in_maps = []
    for c in range(N_CORES):
        if c < 4:
            m = dict(base_f)
            m["xin"] = arrange_x(xf[B * c:B * c + B])
        else:
            m = dict(base_b)
            m["xin"] = arrange_x(xb[B * (c - 4):B * (c - 4) + B])
        in_maps.append(m)
    return in_maps


def kernel(x, W_x_fwd, W_h_fwd, b_fwd, W_x_bwd, W_h_bwd, b_bwd):
    x = np.asarray(x, np.float32)
    assert x.shape == (N, T, D), x.shape
    with_bias = bool(np.any(np.asarray(b_fwd)) or np.any(np.asarray(b_bwd)))
    nc = _get_program(with_bias)
    in_maps = make_in_maps(x, W_x_fwd, W_h_fwd, b_fwd, W_x_bwd, W_h_bwd,
                           b_bwd, with_bias)
    res = bass_utils.run_bass_kernel_spmd(nc, in_maps,
                                          core_ids=list(range(N_CORES)))
    out = np.empty((N, T, 2 * H), np.float32)
    for c in range(N_CORES):
        dec = decode_out(res.results[c]["out"], T)
        if c < 4:
            out[B * c:B * c + B, :, :H] = dec
        else:
            out[B * (c - 4):B * (c - 4) + B, :, H:] = dec[:, ::-1]
    return out
